# revision 13
# baseline (speedup 1.0000x reference)
"""MergeAdapter (moe_routing) Trainium2 Bass kernel.

Reference computation (per instance n):
    wd = sum_k prob[n,k] * w_down[k]   (D, H)     bd = sum_k prob[n,k] * b_down[k]
    wu = sum_k prob[n,k] * w_up[k]     (H, D)     bu = sum_k prob[n,k] * b_up[k]
    out[n] = x[n] + relu(x[n] @ wd.T + bd) @ wu.T + bu

Sharding: data-parallel over the instance dim N=16 -> 2 instances per core on
8 cores; every core holds the full expert banks. No cross-core communication.

Device kernel design (per core, all matmuls bf16 with fp32 PSUM accumulation):
  - expert banks are merged on-chip with fused DVE multiply-add
    (scalar_tensor_tensor: acc = bank_k * prob_k + acc)
  - merged b_down is applied as the per-partition bias of the ACT relu
  - merged b_up rides the second matmul's PSUM accumulation as a K=1
    ones-row matmul; the residual skip-add rides it as an identity matmul
  - hidden_states is supplied both natural (skip-add path) and transposed
    (contraction operand of the first matmul), cast to bf16 host-side
"""
import os
import sys

for _p in ("/opt/trn_rl_repo",):
    if os.path.isdir(_p) and _p not in sys.path:
        sys.path.insert(0, _p)

import ml_dtypes
import numpy as np

import concourse.mybir as mybir
import concourse.tile as tile
from concourse import bacc
from concourse.bass_utils import run_bass_kernel_spmd

N, S, H, K, D = 16, 2048, 1024, 8, 256
NCORES = 8
NPC = N // NCORES          # instances per core
IC = H // 128              # contraction chunks of the first matmul
OC = D // 128              # o-chunks (partition tiles of the bottleneck dim)
SCW = 512                  # first-matmul psum free-dim width
HCW = 512                  # second-matmul psum free-dim width
ST = S // 128              # s-tiles (output partition tiles)

BF16 = mybir.dt.float16   # 2-byte compute dtype (fp16: O(1) data, mantissa > range)
F32 = mybir.dt.float32
bf16 = np.float16

_CACHE: dict = {}


def _emit(nc, tc, tens, repeat=1, loop_t=None):
    (xT_d, xn_d, wdT_d, wuT_d, bd_d, bu_d, pb_d, pkn_d, eye_d, ones_d, out_d) = tens
    with (
        tc.tile_pool(name="consts", bufs=1) as consts,
        tc.tile_pool(name="banks", bufs=K) as banks,
        tc.tile_pool(name="work", bufs=1) as work,
        tc.tile_pool(name="xtp", bufs=1) as xtp,
        tc.tile_pool(name="stream", bufs=4) as stream,
        tc.tile_pool(name="ps1", bufs=2, space="PSUM") as ps1p,
        tc.tile_pool(name="ps2", bufs=4, space="PSUM") as ps2p,
        tc.tile_pool(name="pst", bufs=1, space="PSUM") as pstiny,
    ):
        pb_t = consts.tile([128, NPC * K], F32, tag="pb")
        pkn_t = consts.tile([K, NPC], F32, tag="pkn")
        bd_t = consts.tile([K, D], F32, tag="bd")
        bu_t = consts.tile([K, H], F32, tag="bu")
        eye_t = consts.tile([128, 128], BF16, tag="eye")
        ones_t = consts.tile([1, 128], BF16, tag="ones")
        nc.sync.dma_start(pb_t[:], pb_d.ap())
        nc.sync.dma_start(pkn_t[:], pkn_d.ap())
        nc.sync.dma_start(bd_t[:], bd_d.ap())
        nc.sync.dma_start(bu_t[:], bu_d.ap())
        nc.sync.dma_start(eye_t[:], eye_d.ap())
        nc.sync.dma_start(ones_t[:], ones_d.ap())

        if loop_t is not None:
            # timing mode: run the whole body loop_t times on-device
            import contextlib
            loop_cm = tc.For_i(0, loop_t, 1, hint_engines=tuple(
                getattr(mybir.EngineType, e)
                for e in ("PE", "DVE", "Activation", "SP", "Pool")))
        else:
            import contextlib
            loop_cm = contextlib.nullcontext()

        with loop_cm:
          for rep in range(repeat):
            # ---- merged biases ----
            # mbd[:, oc*NPC+n] = per-partition merged b_down of (o-chunk oc, inst n)
            mbd_t = work.tile([128, OC * NPC], F32, tag="mbd")
            mbu_t = [work.tile([1, H], BF16, tag=f"mbu{n}", name=f"mbu{n}")
                     for n in range(NPC)]
            for oc in range(OC):
                psbd = pstiny.tile([128, NPC], F32, tag="psbd")
                nc.tensor.matmul(psbd[:], bd_t[:, oc * 128:(oc + 1) * 128], pkn_t[:])
                nc.vector.tensor_copy(mbd_t[:, oc * NPC:(oc + 1) * NPC], psbd[:])
            for n in range(NPC):
                for hc in range(H // HCW):
                    psbu = pstiny.tile([1, HCW], F32, tag="psbu")
                    nc.tensor.matmul(psbu[:], pkn_t[:, n:n + 1],
                                     bu_t[:, hc * HCW:(hc + 1) * HCW])
                    nc.scalar.copy(mbu_t[n][0:1, hc * HCW:(hc + 1) * HCW], psbu[:])

            # ---- DMA: wd banks -> xT (all instances) -> wu banks ----
            wd_banks, wu_banks = [], []
            for k in range(K):
                bk = banks.tile([128, IC, D], BF16, tag="bank", name=f"wdb{k}")
                nc.sync.dma_start(
                    bk[:], wdT_d.ap()[k].rearrange("(c p) o -> p c o", p=128))
                wd_banks.append(bk)
            xt = {}
            for n in range(NPC):
                for ic in range(IC):
                    t = xtp.tile([128, S], BF16, tag=f"xt{n}_{ic}", name=f"xt{n}_{ic}")
                    nc.sync.dma_start(
                        t[:], xT_d.ap()[n, ic * 128:(ic + 1) * 128, :])
                    xt[(n, ic)] = t
            for k in range(K):
                bk = banks.tile([128, OC, H], BF16, tag="bank", name=f"wub{k}")
                nc.sync.dma_start(
                    bk[:], wuT_d.ap()[k].rearrange("(c p) h -> p c h", p=128))
                wu_banks.append(bk)

            # ---- merge chains on DVE, split by output half so the first
            # ---- dependent matmul group unblocks after half a chain
            wdm = [work.tile([128, IC, D], BF16, tag=f"wdm{n}", name=f"wdm{n}")
                   for n in range(NPC)]
            wum = [work.tile([128, OC, H], BF16, tag=f"wum{n}", name=f"wum{n}")
                   for n in range(NPC)]
            for n in range(NPC):
                for oc in range(OC):    # wdm half = o columns used by mm1 group oc
                    dst = wdm[n][:, :, oc * 128:(oc + 1) * 128]
                    for k in range(K):
                        src = wd_banks[k][:, :, oc * 128:(oc + 1) * 128]
                        sc_ap = pb_t[:, n * K + k:n * K + k + 1]
                        if k == 0:
                            nc.vector.tensor_scalar_mul(dst, src, sc_ap)
                        else:
                            nc.vector.scalar_tensor_tensor(
                                dst, src, sc_ap, dst,
                                mybir.AluOpType.mult, mybir.AluOpType.add)
            for n in range(NPC):
                for hc in range(H // HCW):  # wum half = h columns of mm2 group hc
                    dst = wum[n][:, :, hc * HCW:(hc + 1) * HCW]
                    for k in range(K):
                        src = wu_banks[k][:, :, hc * HCW:(hc + 1) * HCW]
                        sc_ap = pb_t[:, n * K + k:n * K + k + 1]
                        if k == 0:
                            nc.vector.tensor_scalar_mul(dst, src, sc_ap)
                        else:
                            nc.vector.scalar_tensor_tensor(
                                dst, src, sc_ap, dst,
                                mybir.AluOpType.mult, mybir.AluOpType.add)

            # ---- matmul 1 + relu(. + bd), both instances ----
            relu1 = {}
            for n in range(NPC):
                for oc in range(OC):
                    relu1[(n, oc)] = work.tile(
                        [128, S], BF16, tag=f"relu{oc}_{n}", name=f"relu{oc}_{n}")
                for sc in range(S // SCW):
                    for oc in range(OC):
                        p1 = ps1p.tile([128, SCW], F32, tag="ps1")
                        for ic in range(IC):
                            nc.tensor.matmul(
                                p1[:],
                                wdm[n][:, ic, oc * 128:(oc + 1) * 128],
                                xt[(n, ic)][:, sc * SCW:(sc + 1) * SCW],
                                start=(ic == 0), stop=(ic == IC - 1))
                        nc.scalar.activation(
                            relu1[(n, oc)][:, sc * SCW:(sc + 1) * SCW], p1[:],
                            mybir.ActivationFunctionType.Relu,
                            bias=mbd_t[:, oc * NPC + n:oc * NPC + n + 1], scale=1.0)

            # ---- matmul 2 + bias + skip-add, then store ----
            for n in range(NPC):
                for st in range(ST):
                    xn_t = stream.tile([128, H], BF16, tag="xn")
                    nc.sync.dma_start(
                        xn_t[:], xn_d.ap()[n, st * 128:(st + 1) * 128, :])
                    ob = stream.tile([128, H], BF16, tag="ob")
                    for hc in range(H // HCW):
                        p2 = ps2p.tile([128, HCW], F32, tag="ps2")
                        for oc in range(OC):
                            nc.tensor.matmul(
                                p2[:],
                                relu1[(n, oc)][:, st * 128:(st + 1) * 128],
                                wum[n][:, oc, hc * HCW:(hc + 1) * HCW],
                                start=(oc == 0), stop=False)
                        nc.tensor.matmul(
                            p2[:], ones_t[:], mbu_t[n][0:1, hc * HCW:(hc + 1) * HCW],
                            start=False, stop=False)
                        nc.tensor.matmul(
                            p2[:], eye_t[:], xn_t[:, hc * HCW:(hc + 1) * HCW],
                            start=False, stop=True)
                        nc.scalar.copy(ob[:, hc * HCW:(hc + 1) * HCW], p2[:])
                    nc.gpsimd.dma_start(
                        out_d.ap()[n, st * 128:(st + 1) * 128, :], ob[:])


def build(repeat=1, loop_t=None):
    """Build and compile the per-core NEFF. Cached per (repeat, loop_t)."""
    key = (repeat, loop_t)
    if key in _CACHE:
        return _CACHE[key]
    nc = bacc.Bacc("TRN2", target_bir_lowering=False, debug=False,
                   num_devices=NCORES)
    tens = (
        nc.dram_tensor("xT", [NPC, H, S], BF16, kind="ExternalInput"),
        nc.dram_tensor("xn", [NPC, S, H], BF16, kind="ExternalInput"),
        nc.dram_tensor("wdT", [K, H, D], BF16, kind="ExternalInput"),
        nc.dram_tensor("wuT", [K, D, H], BF16, kind="ExternalInput"),
        nc.dram_tensor("bd", [K, D], F32, kind="ExternalInput"),
        nc.dram_tensor("bu", [K, H], F32, kind="ExternalInput"),
        nc.dram_tensor("pb", [128, NPC * K], F32, kind="ExternalInput"),
        nc.dram_tensor("pkn", [K, NPC], F32, kind="ExternalInput"),
        nc.dram_tensor("eye", [128, 128], BF16, kind="ExternalInput"),
        nc.dram_tensor("ones", [1, 128], BF16, kind="ExternalInput"),
        nc.dram_tensor("out", [NPC, S, H], BF16, kind="ExternalOutput"),
    )
    with tile.TileContext(nc) as tc:
        _emit(nc, tc, tens, repeat=repeat, loop_t=loop_t)
    nc.compile()
    _CACHE[key] = nc
    return nc


def make_in_maps(hidden_states, prob, w_down, b_down, w_up, b_up):
    """Shard + lay out the full inputs for the 8 cores."""
    hs = np.asarray(hidden_states, dtype=np.float32)
    prob = np.asarray(prob, dtype=np.float32)
    wdT = np.ascontiguousarray(
        np.asarray(w_down, dtype=np.float32).transpose(0, 2, 1)).astype(bf16)
    wuT = np.ascontiguousarray(
        np.asarray(w_up, dtype=np.float32).transpose(0, 2, 1)).astype(bf16)
    bd = np.ascontiguousarray(np.asarray(b_down, dtype=np.float32))
    bu = np.ascontiguousarray(np.asarray(b_up, dtype=np.float32))
    eye = np.eye(128, dtype=np.float32).astype(bf16)
    ones = np.ones((1, 128), dtype=np.float32).astype(bf16)
    in_maps = []
    for c in range(NCORES):
        shard = hs[c * NPC:(c + 1) * NPC]
        p_shard = prob[c * NPC:(c + 1) * NPC]           # (NPC, K)
        in_maps.append({
            "xT": np.ascontiguousarray(shard.transpose(0, 2, 1)).astype(bf16),
            "xn": shard.astype(bf16),
            "wdT": wdT,
            "wuT": wuT,
            "bd": bd,
            "bu": bu,
            "pb": np.tile(p_shard.reshape(1, NPC * K), (128, 1)).astype(np.float32),
            "pkn": np.ascontiguousarray(p_shard.T),
            "eye": eye,
            "ones": ones,
        })
    return in_maps


def kernel(hidden_states, prob, w_down, b_down, w_up, b_up):
    nc = build()
    in_maps = make_in_maps(hidden_states, prob, w_down, b_down, w_up, b_up)
    res = run_bass_kernel_spmd(nc, in_maps, list(range(NCORES)))
    out = np.concatenate([res.results[c]["out"] for c in range(NCORES)], axis=0)
    return np.ascontiguousarray(out.reshape(N, S, H).astype(np.float32))
